# revision 1
# baseline (speedup 1.0000x reference)
"""Trainium2 Bass kernel for nn_ComplexAttention (sparse single-query attention
over H history slots with complex-valued channels).

Key algebraic restructure (exact, not an approximation):
  reference computes   k = hist @ wk ; v = hist @ wv        (412 GFLOP)
  but scores[bt,h] = q[bt]·k[bt,h] = hist[bt,h]·(q @ wk^T)[bt]
  and  ctx[bt]     = sum_h attn[bt,h]*v[bt,h]
                   = (sum_h attn[bt,h]*hist[bt,h]) @ wv + bv   (sum attn = 1)
  so the two huge projection GEMMs collapse into two streaming passes over
  hist (one fused multiply-reduce for scores, one fused multiply-accumulate
  for the weighted mean) plus three small GEMMs (q, p=q@wk^T, ctx=m@wv).

Sharding: data-parallel over the flattened (B,T)=1024 rows, 128 rows/core
on 8 cores. Weights replicated. No collectives.

Per-core device program (hist resident in SBUF as [bt=128 part, H, 2D] bf16):
  q   = cur_cat @ wq                 (PE, lhsT = host-transposed cur_cat)
  qT  = transpose(q)                 (PE transposes, 8x 128x128)
  p   = q @ wk^T                     (PE, lhsT = qT, rhs = host-transposed wk)
  scores[:,h] = sum_c hist[:,h,:]*p  (DVE tensor_tensor_reduce, fused)
  attn = softmax(scores * conf/32)   (DVE + ACT exp)
  m   = sum_h attn[:,h]*hist[:,h,:]  (DVE scalar_tensor_tensor, fused mul-add)
  ctx = m @ wv                       (PE, lhsT = transpose(m))
  out = cur_cat + 0.1*ctx            (DVE fused, then DMA out)
"""

import numpy as np
import ml_dtypes

B, T, H, D = 4, 256, 32, 1024
C2 = 2 * D          # 2048
NCORES = 8
RPC = (B * T) // NCORES   # 128 rows (b,t) per core
P = 128

BF16 = ml_dtypes.bfloat16

_CACHE: dict = {}


def _build_program(has_bq: bool, has_bk: bool, has_bv: bool, m_bf16: bool):
    import concourse.bass as bass
    import concourse.mybir as mybir
    import concourse.tile as tile
    from concourse import bacc
    from concourse.masks import make_identity

    dt = mybir.dt
    f32, bf16 = dt.float32, dt.bfloat16
    mult = mybir.AluOpType.mult
    add = mybir.AluOpType.add
    Ax = mybir.AxisListType

    nc = bacc.Bacc(
        "TRN2",
        target_bir_lowering=False,
        debug=False,
        enable_asserts=False,
        num_devices=NCORES,
    )

    hist_d = nc.dram_tensor("hist", [RPC, H, C2], bf16, kind="ExternalInput").ap()
    curT_d = nc.dram_tensor("curT", [C2, RPC], bf16, kind="ExternalInput").ap()
    cur_d = nc.dram_tensor("cur", [RPC, C2], f32, kind="ExternalInput").ap()
    conf_d = nc.dram_tensor("conf", [RPC, 1], f32, kind="ExternalInput").ap()
    wq_d = nc.dram_tensor("wq", [C2, D], bf16, kind="ExternalInput").ap()
    wkT_d = nc.dram_tensor("wkT", [D, C2], bf16, kind="ExternalInput").ap()
    wv_d = nc.dram_tensor("wv", [C2, C2], bf16, kind="ExternalInput").ap()
    if has_bq:
        bq_d = nc.dram_tensor("bq", [1, D], bf16, kind="ExternalInput").ap()
    if has_bk:
        bk_d = nc.dram_tensor("bk", [1, D], bf16, kind="ExternalInput").ap()
    if has_bv:
        bv_d = nc.dram_tensor("bv", [1, C2], bf16, kind="ExternalInput").ap()
    out_d = nc.dram_tensor("out", [RPC, C2], f32, kind="ExternalOutput").ap()

    KQ = C2 // P   # 16 k-tiles over the 2048 contraction dim
    KD = D // P    # 8 k-tiles over the 1024 contraction dim
    NQ = D // 512  # 2 n-chunks for q
    NC_ = C2 // 512  # 4 n-chunks for p/ctx
    m_dt = bf16 if m_bf16 else f32

    with tile.TileContext(nc) as tc:
        with (
            tc.tile_pool(name="const", bufs=1) as constp,
            tc.tile_pool(name="histp", bufs=1) as histp,
            tc.tile_pool(name="wstream", bufs=2) as wsp,
            tc.tile_pool(name="work", bufs=1) as workp,
            tc.tile_pool(name="pbig", bufs=1, space="PSUM") as pbig,
            tc.tile_pool(name="ptp", bufs=2, space="PSUM") as ptp,
        ):
            ident = constp.tile([P, P], f32)
            make_identity(nc, ident)

            # ---- resident inputs ----
            hist_sb = histp.tile([P, H, C2], bf16)
            for g in range(8):
                nc.sync.dma_start(
                    hist_sb[:, 4 * g : 4 * (g + 1), :],
                    hist_d[:, 4 * g : 4 * (g + 1), :],
                )
            curT_sb = constp.tile([P, KQ, P], bf16)
            nc.sync.dma_start(
                curT_sb[:], curT_d.rearrange("(ko p) bt -> p ko bt", p=P)
            )
            cur_sb = constp.tile([P, C2], f32)
            nc.sync.dma_start(cur_sb[:], cur_d)
            conf_sb = constp.tile([P, 1], f32)
            nc.sync.dma_start(conf_sb[:], conf_d)

            # ---- q = cur_cat @ wq  -> psum_q [128, 1024] ----
            psum_q_full = pbig.tile([P, C2], f32, tag="big", name="psum_q")
            psum_q = psum_q_full[:, :D]
            for k in range(KQ):
                wq_k = wsp.tile([P, D], bf16, tag="wq")
                nc.sync.dma_start(wq_k[:], wq_d[k * P : (k + 1) * P, :])
                for n in range(NQ):
                    nc.tensor.matmul(
                        psum_q[:, n * 512 : (n + 1) * 512],
                        lhsT=curT_sb[:, k, :],
                        rhs=wq_k[:, n * 512 : (n + 1) * 512],
                        start=(k == 0),
                        stop=(k == KQ - 1) and not has_bq,
                    )
            if has_bq:
                bq_sb = constp.tile([1, D], bf16)
                nc.sync.dma_start(bq_sb[:], bq_d)
                ones1 = constp.tile([1, P], bf16)
                nc.vector.memset(ones1[:], 1.0)
                for n in range(NQ):
                    nc.tensor.matmul(
                        psum_q[:, n * 512 : (n + 1) * 512],
                        lhsT=ones1[:],
                        rhs=bq_sb[:, n * 512 : (n + 1) * 512],
                        start=False,
                        stop=(n == NQ - 1),
                    )
            q_sb = workp.tile([P, D], f32)
            nc.scalar.copy(q_sb[:], psum_q[:])

            # ---- qT via PE transposes (fp32 in/out, cast to bf16 on copy-back) ----
            qT_sb = workp.tile([P, KD, P], bf16)
            for dk in range(KD):
                pt = ptp.tile([P, P], f32, tag="tp")
                nc.tensor.transpose(pt[:], q_sb[:, dk * P : (dk + 1) * P], ident[:])
                nc.scalar.copy(qT_sb[:, dk, :], pt[:])

            # ---- p = q @ wk^T -> psum_p [128, 2048] ----
            psum_p = pbig.tile([P, C2], f32, tag="big")
            for dk in range(KD):
                wkT_k = wsp.tile([P, C2], bf16, tag="w2048")
                nc.sync.dma_start(wkT_k[:], wkT_d[dk * P : (dk + 1) * P, :])
                for n in range(NC_):
                    nc.tensor.matmul(
                        psum_p[:, n * 512 : (n + 1) * 512],
                        lhsT=qT_sb[:, dk, :],
                        rhs=wkT_k[:, n * 512 : (n + 1) * 512],
                        start=(dk == 0),
                        stop=(dk == KD - 1),
                    )
            p_sb = workp.tile([P, C2], bf16)
            nc.scalar.copy(p_sb[:], psum_p[:])

            # ---- fused online scores + weighted-mean pass ----
            # scores[:, h] = conf/sqrt(d) * sum_c hist[:,h,:] * p.
            # DVE does the elementwise product (bf16 2x); ScalarE does the
            # free-dim sum via activation(Copy, accum_out) with the confidence
            # scale folded into the per-partition activation scale, then the
            # exp. Logits are bounded (weights ~0.02, scale 1/32, conf<=1) so
            # softmax needs no max subtraction: e_h = exp(s_h) directly, and
            # m accumulates e_h-weighted history on DVE while ACT reduces
            # later heads. Normalization by 1/sum(e) happens once at the end.
            scores = workp.tile([P, H], f32)
            attn = workp.tile([P, H], f32)  # holds e_h = exp(s_h)
            Copy = mybir.ActivationFunctionType.Copy
            Exp = mybir.ActivationFunctionType.Exp
            m_sb = workp.tile([P, C2], m_dt)

            if has_bk:
                bk_rep = constp.tile([P, D], bf16)
                nc.sync.dma_start(bk_rep[:], bk_d.to_broadcast([P, D]))
                qbk_tmp = workp.tile([P, D], bf16)
                qbk = workp.tile([P, 1], f32)
                nc.vector.tensor_tensor(qbk_tmp[:], q_sb[:], bk_rep[:], mult)
                nc.scalar.activation(
                    qbk_tmp[:], qbk_tmp[:], Copy,
                    scale=conf_sb[:, 0:1], accum_out=qbk[:],
                )

            for h in range(H):
                tmp = workp.tile([P, C2], bf16, tag="ttr_tmp", bufs=3)
                nc.vector.tensor_tensor(tmp[:], hist_sb[:, h, :], p_sb[:], mult)
                nc.scalar.activation(
                    tmp[:], tmp[:], Copy,
                    scale=conf_sb[:, 0:1],
                    accum_out=scores[:, h : h + 1],
                )
                if has_bk:
                    nc.vector.tensor_tensor(
                        scores[:, h : h + 1], scores[:, h : h + 1], qbk[:], add
                    )
                # e_h = exp(s_h), tiny [128,1] ACT op
                nc.scalar.activation(
                    attn[:, h : h + 1], scores[:, h : h + 1], Exp
                )
                # m (+)= e_h * hist_h on DVE, overlapped with ACT's next reduce
                if h == 0:
                    nc.vector.tensor_scalar_mul(
                        m_sb[:], hist_sb[:, 0, :], attn[:, 0:1]
                    )
                else:
                    nc.vector.scalar_tensor_tensor(
                        out=m_sb[:],
                        in0=hist_sb[:, h, :],
                        scalar=attn[:, h : h + 1],
                        in1=m_sb[:],
                        op0=mult,
                        op1=add,
                    )

            # normalize: m *= 1/sum_h e_h, folded into the fp32 copy for mT
            ssum = workp.tile([P, 1], f32)
            nc.vector.reduce_sum(ssum[:], attn[:], axis=Ax.X)
            rec = workp.tile([P, 1], f32)
            nc.vector.reciprocal(rec[:], ssum[:])
            if m_bf16:
                m_f = workp.tile([P, C2], f32)
                nc.vector.tensor_scalar_mul(m_f[:], m_sb[:], rec[:, 0:1])
            else:
                nc.vector.tensor_scalar_mul(m_sb[:], m_sb[:], rec[:, 0:1])
                m_f = m_sb

            # ---- mT via PE transposes (fp32 in/out, cast to bf16 on copy-back) ----
            mT_sb = workp.tile([P, KQ, P], bf16)
            for ck in range(KQ):
                pt2 = ptp.tile([P, P], f32, tag="tp")
                nc.tensor.transpose(pt2[:], m_f[:, ck * P : (ck + 1) * P], ident[:])
                nc.scalar.copy(mT_sb[:, ck, :], pt2[:])

            # ---- ctx = m @ wv -> psum_ctx [128, 2048] ----
            psum_ctx = pbig.tile([P, C2], f32, tag="big")
            for ck in range(KQ):
                wv_k = wsp.tile([P, C2], bf16, tag="w2048")
                nc.sync.dma_start(wv_k[:], wv_d[ck * P : (ck + 1) * P, :])
                for n in range(NC_):
                    nc.tensor.matmul(
                        psum_ctx[:, n * 512 : (n + 1) * 512],
                        lhsT=mT_sb[:, ck, :],
                        rhs=wv_k[:, n * 512 : (n + 1) * 512],
                        start=(ck == 0),
                        stop=(ck == KQ - 1) and not has_bv,
                    )
            if has_bv:
                bv_sb = constp.tile([1, C2], bf16)
                nc.sync.dma_start(bv_sb[:], bv_d)
                ones1b = constp.tile([1, P], bf16)
                nc.vector.memset(ones1b[:], 1.0)
                for n in range(NC_):
                    nc.tensor.matmul(
                        psum_ctx[:, n * 512 : (n + 1) * 512],
                        lhsT=ones1b[:],
                        rhs=bv_sb[:, n * 512 : (n + 1) * 512],
                        start=False,
                        stop=(n == NC_ - 1),
                    )

            # ---- out = cur + 0.1 * ctx  (in-place into cur_sb) ----
            nc.vector.scalar_tensor_tensor(
                out=cur_sb[:],
                in0=psum_ctx[:],
                scalar=0.1,
                in1=cur_sb[:],
                op0=mult,
                op1=add,
            )
            nc.sync.dma_start(out_d, cur_sb[:])

    nc.compile()
    return nc


def _get_program(flags):
    if flags not in _CACHE:
        _CACHE[flags] = _build_program(*flags)
    return _CACHE[flags]


def kernel(**inputs) -> np.ndarray:
    hist_real = np.asarray(inputs["hist_real"], np.float32)
    hist_imag = np.asarray(inputs["hist_imag"], np.float32)
    cur_real = np.asarray(inputs["cur_real"], np.float32)
    cur_imag = np.asarray(inputs["cur_imag"], np.float32)
    confidence = np.asarray(inputs["confidence"], np.float32)
    wq = np.asarray(inputs["wq"], np.float32)
    bq = np.asarray(inputs["bq"], np.float32)
    wk = np.asarray(inputs["wk"], np.float32)
    bk = np.asarray(inputs["bk"], np.float32)
    wv = np.asarray(inputs["wv"], np.float32)
    bv = np.asarray(inputs["bv"], np.float32)

    has_bq = bool(np.any(bq))
    has_bk = bool(np.any(bk))
    has_bv = bool(np.any(bv))
    flags = (has_bq, has_bk, has_bv, False)
    nc = _get_program(flags)

    BT = B * T
    hr = hist_real.reshape(BT, H, D)
    hi = hist_imag.reshape(BT, H, D)
    cur_cat = np.concatenate(
        [cur_real.reshape(BT, D), cur_imag.reshape(BT, D)], axis=-1
    )
    conf_scaled = (confidence.reshape(BT, 1) * (D ** -0.5)).astype(np.float32)
    wq_b = np.ascontiguousarray(wq, dtype=BF16)
    wkT_b = np.ascontiguousarray(wk.T, dtype=BF16)
    wv_b = np.ascontiguousarray(wv, dtype=BF16)

    in_maps = []
    for c in range(NCORES):
        sl = slice(c * RPC, (c + 1) * RPC)
        hist_c = np.empty((RPC, H, C2), dtype=BF16)
        hist_c[:, :, :D] = hr[sl]
        hist_c[:, :, D:] = hi[sl]
        cur_c = np.ascontiguousarray(cur_cat[sl])
        m = {
            "hist": hist_c,
            "curT": np.ascontiguousarray(cur_c.T, dtype=BF16),
            "cur": cur_c,
            "conf": np.ascontiguousarray(conf_scaled[sl]),
            "wq": wq_b,
            "wkT": wkT_b,
            "wv": wv_b,
        }
        if has_bq:
            m["bq"] = np.ascontiguousarray(bq.reshape(1, D), dtype=BF16)
        if has_bk:
            m["bk"] = np.ascontiguousarray(bk.reshape(1, D), dtype=BF16)
        if has_bv:
            m["bv"] = np.ascontiguousarray(bv.reshape(1, C2), dtype=BF16)
        in_maps.append(m)

    from concourse import bass_utils

    res = bass_utils.run_bass_kernel_spmd(
        nc, in_maps, core_ids=list(range(NCORES))
    )
    out_cat = np.concatenate([r["out"] for r in res.results], axis=0)  # [1024, 2048]
    out = np.empty((BT, D), dtype=np.complex64)
    out.real = out_cat[:, :D]
    out.imag = out_cat[:, D:]
    return out.reshape(B, T, D)



# revision 11
# speedup vs baseline: 2.1807x; 2.1807x over previous
"""Trainium2 Bass kernel for nn_ComplexAttention (single-query attention over
H history slots with complex-valued channels).

Algebraic restructure (exact):
  reference: k = hist@wk ; v = hist@wv  (412 GFLOP of GEMM)
  but scores[bt,h] = q[bt]·k[bt,h] = hist[bt,h,:]·p[bt,:],  p = cur_cat@(wq@wk^T)
  and  ctx[bt]     = (sum_h e_h hist[bt,h]) @ wv / sum_h e_h
  W = wq@wk^T is precomputed on host (weight-only), so the device runs ONE
  input GEMM (p), a streaming softmax pass over hist, and ONE output GEMM.

Sharding: data-parallel over the flattened (B,T)=1024 rows, 128 rows/core
on 8 cores. Weights replicated (fp8, scaled). No collectives.

Per-core schedule (target ~115us, from 256us baseline):
  DMA order: conf/curT -> hist chunk0 -> W tiles (16) -> hist chunks -> cur -> wv
  PE:  p = curT8.T @ W8 (fp8, k-outer; chases the W tile DMAs)
  loop h=0..31 (3-engine split):
    DVE:  tmp_h = hist_h * p              (bf16 TT, 2x mode)
    ACT:  sraw_h = conf * sum_c tmp_h     (Copy+scale+accum), e_h = exp(sraw_h)
    DVE/Pool: tmp2_h = e_h * hist_h       (tensor_scalar 4x on DVE; ~25 of 32
                                           heads on the otherwise-idle Pool)
    DVE:  m += tmp2_h                     (bf16 TT add, 2x mode)
  tail: rec = 1/sum e; mT8 = PE-transpose(m)*0.25 -> fp8
        ctx = mT8.T @ wv8 (n-outer, resident wv8)
        out_n = cur_n + (0.1*rec/(SCALE_WV*SCALE_M)) * ctx_n ; DMA per chunk
"""

import numpy as np
import ml_dtypes

B, T, H, D = 4, 256, 32, 1024
C2 = 2 * D          # 2048
NCORES = 8
RPC = (B * T) // NCORES   # 128 rows (b,t) per core
P = 128

BF16 = ml_dtypes.bfloat16
FP8 = ml_dtypes.float8_e4m3fn

SCALE_W = 128.0    # W fp8 scale (W entries ~0.013 std; lift out of subnormals)
SCALE_WV = 64.0    # wv fp8 scale
SCALE_M = 0.25     # mT fp8 scale (keep unnormalized m under e4m3 max 448)

_CACHE: dict = {}


def _build_program(has_bq: bool, has_bk: bool, _unused1=False, _unused2=False):
    import concourse.bass as bass
    import concourse.mybir as mybir
    import concourse.tile as tile
    from concourse import bacc
    from concourse.masks import make_identity

    dt = mybir.dt
    f32, bf16, f8 = dt.float32, dt.bfloat16, dt.float8e4
    mult = mybir.AluOpType.mult
    add = mybir.AluOpType.add
    Ax = mybir.AxisListType
    Copy = mybir.ActivationFunctionType.Copy
    Exp = mybir.ActivationFunctionType.Exp

    nc = bacc.Bacc(
        "TRN2",
        target_bir_lowering=False,
        debug=False,
        enable_asserts=False,
        num_devices=NCORES,
    )

    hist_d = nc.dram_tensor("hist", [RPC, H, C2], bf16, kind="ExternalInput").ap()
    curT_d = nc.dram_tensor("curT", [C2, RPC], f8, kind="ExternalInput").ap()
    cur_d = nc.dram_tensor("cur", [RPC, C2], f32, kind="ExternalInput").ap()
    conf_d = nc.dram_tensor("conf", [RPC, 1], f32, kind="ExternalInput").ap()
    w_d = nc.dram_tensor("w", [C2, C2], f8, kind="ExternalInput").ap()
    wv_d = nc.dram_tensor("wv", [C2, C2], f8, kind="ExternalInput").ap()
    if has_bq:
        r_d = nc.dram_tensor("r", [1, C2], f8, kind="ExternalInput").ap()
    if has_bk:
        u_d = nc.dram_tensor("u", [C2, 1], f8, kind="ExternalInput").ap()
    out_d = nc.dram_tensor("out", [RPC, C2], f32, kind="ExternalOutput").ap()

    KT = C2 // P      # 16 k-tiles over the 2048 contraction dim
    NCH = C2 // 512   # 4 n-chunks of 512
    HCH = 2           # heads per hist DMA chunk
    NHBUF = 11        # hist ring: 11 chunks x 2 heads resident
    # heads whose e_h*hist_h product runs on DVE (4x tensor_scalar); the rest
    # run on the otherwise-idle Pool engine with their m-add deferred 3 heads
    # so Pool's 2.94us latency never gates the serial m accumulation chain.
    DVE_T3 = {6, 13, 20, 27, 28, 29, 30, 31}

    with tile.TileContext(nc) as tc:
        with (
            tc.tile_pool(name="const", bufs=1) as constp,
            tc.tile_pool(name="histp", bufs=NHBUF) as histp,
            tc.tile_pool(name="wstream", bufs=4) as wsp,
            tc.tile_pool(name="work", bufs=1) as workp,
            tc.tile_pool(name="tmpp", bufs=4) as tmpp,
            tc.tile_pool(name="tmp2p", bufs=6) as tmp2p,
            tc.tile_pool(name="pbig", bufs=1, space="PSUM") as pbig,
            tc.tile_pool(name="ptp", bufs=2, space="PSUM") as ptp,
        ):
            ident = constp.tile([P, P], bf16)
            make_identity(nc, ident)

            # ---- DMA: small inputs, first hist chunk, then W tiles ----
            conf_sb = constp.tile([P, 1], f32)
            nc.sync.dma_start(conf_sb[:], conf_d)
            curT_sb = constp.tile([P, KT, P], f8)
            nc.sync.dma_start(curT_sb[:], curT_d.rearrange("(k p) bt -> p k bt", p=P))
            if has_bq:
                r_sb = constp.tile([1, C2], f8)
                nc.sync.dma_start(r_sb[:], r_d)
                ones1 = constp.tile([1, P], f8)
                nc.vector.memset(ones1[:], 1.0)
            if has_bk:
                u_sb = constp.tile([P, KT, 1], f8)
                nc.sync.dma_start(u_sb[:], u_d.rearrange("(k p) one -> p k one", p=P))

            hist_t = []
            for c in range(H // HCH):
                ht = histp.tile([P, HCH, C2], bf16, tag="hist")
                hist_t.append(ht)

            # ---- p = cur_cat @ W  (fp8 DoubleRow pairs, chases W DMAs) ----
            DR = mybir.MatmulPerfMode.DoubleRow
            psum_p = pbig.tile([P, C2], f32, tag="big", name="psum_p")
            if has_bk:
                psum_qbk = ptp.tile([P, 1], f32, tag="qbk", bufs=1)
            for j in range(KT // 2):
                w_j = wsp.tile([P, 2, C2], f8, tag="w")
                nc.sync.dma_start(
                    w_j[:],
                    w_d[j * 2 * P : (j + 1) * 2 * P, :].rearrange(
                        "(i p) c -> p i c", p=P
                    ),
                )
                for n in range(NCH):
                    nc.tensor.matmul(
                        psum_p[:, n * 512 : (n + 1) * 512],
                        lhsT=curT_sb[:, 2 * j : 2 * j + 2, :],
                        rhs=w_j[:, :, n * 512 : (n + 1) * 512],
                        start=(j == 0),
                        stop=(j == KT // 2 - 1) and not has_bq,
                        perf_mode=DR,
                    )
                if has_bk:
                    for i in range(2):
                        nc.tensor.matmul(
                            psum_qbk[:],
                            lhsT=curT_sb[:, 2 * j + i, :],
                            rhs=u_sb[:, 2 * j + i, :],
                            start=(j == 0 and i == 0),
                            stop=(j == KT // 2 - 1 and i == 1),
                        )
            if has_bq:
                for n in range(NCH):
                    nc.tensor.matmul(
                        psum_p[:, n * 512 : (n + 1) * 512],
                        lhsT=ones1[:],
                        rhs=r_sb[:, n * 512 : (n + 1) * 512],
                        start=False,
                        stop=(n == NCH - 1),
                    )
            p_sb = workp.tile([P, C2], bf16)
            for n in range(NCH):
                nc.scalar.copy(
                    p_sb[:, n * 512 : (n + 1) * 512],
                    psum_p[:, n * 512 : (n + 1) * 512],
                )

            # ---- remaining DMA: hist chunks, cur, wv (prefetch under loop) ----
            for c in range(H // HCH):
                nc.sync.dma_start(hist_t[c][:], hist_d[:, c * HCH : (c + 1) * HCH, :])
            cur_sb = constp.tile([P, C2], f32)
            nc.sync.dma_start(cur_sb[:], cur_d)
            wv_sb = constp.tile([P, KT, C2], f8)
            nc.sync.dma_start(wv_sb[:], wv_d.rearrange("(k p) c -> p k c", p=P))

            # ---- streaming scores + weighted-sum pass ----
            sraw = workp.tile([P, H], f32)
            evals = workp.tile([P, H], f32)
            m_sb = workp.tile([P, C2], bf16)
            if has_bk:
                bias_e = workp.tile([P, 1], f32)
                nc.vector.tensor_tensor(bias_e[:], conf_sb[:], psum_qbk[:], mult)

            pending = []  # (head, tmp2 tile) Pool products awaiting the m-add
            for h in range(H):
                hist_ap = hist_t[h // HCH][:, h % HCH, :]
                tmp = tmpp.tile([P, C2], bf16, tag="tmp")
                nc.vector.tensor_tensor(tmp[:], hist_ap, p_sb[:], mult)
                nc.scalar.activation(
                    tmp[:], tmp[:], Copy,
                    scale=conf_sb[:, 0:1],
                    accum_out=sraw[:, h : h + 1],
                )
                if has_bk:
                    nc.scalar.activation(
                        evals[:, h : h + 1], sraw[:, h : h + 1], Exp,
                        bias=bias_e[:, 0:1],
                    )
                else:
                    nc.scalar.activation(evals[:, h : h + 1], sraw[:, h : h + 1], Exp)
                if h == 0:
                    nc.vector.tensor_scalar_mul(m_sb[:], hist_ap, evals[:, 0:1])
                elif h in DVE_T3:
                    tmp2 = tmp2p.tile([P, C2], bf16, tag="tmp2")
                    nc.vector.tensor_scalar_mul(tmp2[:], hist_ap, evals[:, h : h + 1])
                    nc.vector.tensor_tensor(m_sb[:], m_sb[:], tmp2[:], add)
                else:
                    tmp2 = tmp2p.tile([P, C2], bf16, tag="tmp2")
                    nc.gpsimd.tensor_scalar_mul(tmp2[:], hist_ap, evals[:, h : h + 1])
                    pending.append((h, tmp2))
                while pending and pending[0][0] <= h - 3:
                    _, t2 = pending.pop(0)
                    nc.vector.tensor_tensor(m_sb[:], m_sb[:], t2[:], add)
            for _, t2 in pending:
                nc.vector.tensor_tensor(m_sb[:], m_sb[:], t2[:], add)

            # ---- normalization scalar: scal = 0.1/(SCALE_WV*SCALE_M) / sum e ----
            ssum = workp.tile([P, 1], f32)
            nc.vector.reduce_sum(ssum[:], evals[:], axis=Ax.X)
            rec = workp.tile([P, 1], f32)
            nc.vector.reciprocal(rec[:], ssum[:])
            scal = workp.tile([P, 1], f32)
            nc.vector.tensor_scalar_mul(scal[:], rec[:], 0.1 / (SCALE_WV * SCALE_M))

            # ---- mT8 via PE transposes (bf16 in, fp8 out scaled) ----
            mT8 = workp.tile([P, KT, P], f8)
            for k in range(KT):
                pt = ptp.tile([P, P], bf16, tag="tp")
                nc.tensor.transpose(pt[:], m_sb[:, k * P : (k + 1) * P], ident[:])
                if k % 2 == 0:
                    nc.scalar.mul(mT8[:, k, :], pt[:], SCALE_M)
                else:
                    nc.vector.tensor_scalar_mul(mT8[:, k, :], pt[:], SCALE_M)

            # ---- ctx = m @ wv (n-outer over resident wv8), fused output ----
            psum_ctx = pbig.tile([P, C2], f32, tag="big", name="psum_ctx")
            for n in range(NCH):
                ns = slice(n * 512, (n + 1) * 512)
                for j in range(KT // 2):
                    nc.tensor.matmul(
                        psum_ctx[:, ns],
                        lhsT=mT8[:, 2 * j : 2 * j + 2, :],
                        rhs=wv_sb[:, 2 * j : 2 * j + 2, ns],
                        start=(j == 0),
                        stop=(j == KT // 2 - 1),
                        perf_mode=DR,
                    )
                nc.vector.scalar_tensor_tensor(
                    out=cur_sb[:, ns],
                    in0=psum_ctx[:, ns],
                    scalar=scal[:, 0:1],
                    in1=cur_sb[:, ns],
                    op0=mult,
                    op1=add,
                )
                nc.sync.dma_start(out_d[:, ns], cur_sb[:, ns])

    nc.compile()
    return nc


def _get_program(flags):
    key = tuple(flags[:2])
    if key not in _CACHE:
        _CACHE[key] = _build_program(*key)
    return _CACHE[key]


def kernel(**inputs) -> np.ndarray:
    hist_real = np.asarray(inputs["hist_real"], np.float32)
    hist_imag = np.asarray(inputs["hist_imag"], np.float32)
    cur_real = np.asarray(inputs["cur_real"], np.float32)
    cur_imag = np.asarray(inputs["cur_imag"], np.float32)
    confidence = np.asarray(inputs["confidence"], np.float32)
    wq = np.asarray(inputs["wq"], np.float32)
    bq = np.asarray(inputs["bq"], np.float32)
    wk = np.asarray(inputs["wk"], np.float32)
    bk = np.asarray(inputs["bk"], np.float32)
    wv = np.asarray(inputs["wv"], np.float32)
    bv = np.asarray(inputs["bv"], np.float32)

    has_bq = bool(np.any(bq))
    has_bk = bool(np.any(bk))
    nc = _get_program((has_bq, has_bk))

    BT = B * T
    hr = hist_real.reshape(BT, H, D)
    hi = hist_imag.reshape(BT, H, D)
    cur_cat = np.concatenate(
        [cur_real.reshape(BT, D), cur_imag.reshape(BT, D)], axis=-1
    )
    conf_scaled = (confidence.reshape(BT, 1) * (D ** -0.5) / SCALE_W).astype(
        np.float32
    )
    # host-precomputed fused score weight: p = cur_cat @ (wq @ wk.T)
    W = wq @ wk.T
    W8 = np.ascontiguousarray(W * SCALE_W, dtype=FP8)
    wv8 = np.ascontiguousarray(wv * SCALE_WV, dtype=FP8)
    cur_dev = cur_cat if not np.any(bv) else cur_cat + 0.1 * bv[None, :]
    if has_bq:
        r8 = np.ascontiguousarray((bq @ wk.T)[None, :] * SCALE_W, dtype=FP8)
    if has_bk:
        u8 = np.ascontiguousarray((wq @ bk)[:, None] * SCALE_W, dtype=FP8)

    in_maps = []
    for c in range(NCORES):
        sl = slice(c * RPC, (c + 1) * RPC)
        hist_c = np.empty((RPC, H, C2), dtype=BF16)
        hist_c[:, :, :D] = hr[sl]
        hist_c[:, :, D:] = hi[sl]
        m = {
            "hist": hist_c,
            "curT": np.ascontiguousarray(cur_cat[sl].T, dtype=FP8),
            "cur": np.ascontiguousarray(cur_dev[sl], dtype=np.float32),
            "conf": np.ascontiguousarray(conf_scaled[sl]),
            "w": W8,
            "wv": wv8,
        }
        if has_bq:
            m["r"] = r8
        if has_bk:
            m["u"] = u8
        in_maps.append(m)

    from concourse import bass_utils

    res = bass_utils.run_bass_kernel_spmd(
        nc, in_maps, core_ids=list(range(NCORES))
    )
    out_cat = np.concatenate([r["out"] for r in res.results], axis=0)  # [1024, 2048]
    out = np.empty((BT, D), dtype=np.complex64)
    out.real = out_cat[:, :D]
    out.imag = out_cat[:, D:]
    return out.reshape(B, T, D)
